# revision 1
# baseline (speedup 1.0000x reference)
"""DLRM pairwise-interaction layer on 8 Trainium2 NeuronCores.

Computes, for each batch row b, the strict upper triangle of the Gram matrix
G_b = E_b @ E_b.T where E_b is (27 features, 128 dims), i.e. the reference

    interactions = einsum("bfd,bgd->bfg", E, E);  out = interactions[:, triu_i, triu_j]

Strategy (pure batch data-parallel, 2048 rows/core):
  * Host pre-transposes E (B, 27, 128) -> ET (128, B, 27) so the contraction
    dim D=128 lies on SBUF partitions for the TensorEngine.
  * SBUF input layout pads each batch row's 27 feature columns to 32, so a
    group of 4 batch rows forms a (128, 128) stationary operand whose four
    27-column strips are 32-aligned.  One fp32 matmul per group with
    moving = the 4x27 unpadded columns (N=108) puts the four wanted 27x27
    Gram blocks at PSUM (32q .. 32q+26, 27q .. 27q+26) — 32-aligned
    partition bases, as the engines require.
  * Diagonal blocks are extracted with strided VectorE/ScalarE copies
    (4 groups per instruction) into a compact SBUF tile and DMA'd out.
  * Host performs the cheap (B, 27, 27) -> (B, 351) triangle gather.
"""

import numpy as np

B = 16384
F = 27
D = 128
NCORES = 8
BLOC = B // NCORES          # 2048 batch rows per core
BCHUNK = 128                # batch rows per pipeline chunk
NCHUNK = BLOC // BCHUNK     # 16
NGRP = BCHUNK // 4          # 32 matmul groups (4 rows each) per chunk
NBANK = NGRP // 4           # 8 psum banks per chunk (4 groups per bank)

_TRIU_I, _TRIU_J = np.triu_indices(F, k=1)

_compiled = None


def _build():
    import concourse.bacc as bacc
    import concourse.mybir as mybir
    from concourse.tile import TileContext

    f32 = mybir.dt.float32
    nc = bacc.Bacc(None, target_bir_lowering=False)

    et = nc.dram_tensor("et", [D, BLOC, F], f32, kind="ExternalInput")
    y = nc.dram_tensor("y", [D, NCHUNK, NGRP, F], f32, kind="ExternalOutput")

    with TileContext(nc) as tc:
        with (
            tc.tile_pool(name="inp", bufs=3) as inp,
            tc.tile_pool(name="outp", bufs=3) as outp,
            tc.tile_pool(name="psum", bufs=8, space="PSUM") as psum,
        ):
            for c in range(NCHUNK):
                # (128, group, row-in-group, 32): 27 real cols padded to 32
                in_t = inp.tile([D, NGRP, 4, 32], f32)
                nc.sync.dma_start(
                    in_t[:, :, :, 0:F],
                    et[:, c * BCHUNK:(c + 1) * BCHUNK, :].rearrange(
                        "p (g r) f -> p g r f", r=4
                    ),
                )
                out_t = outp.tile([D, NGRP, F], f32)
                for bk in range(NBANK):
                    ps = psum.tile([D, 4, 4 * F], f32)
                    for s in range(4):
                        g = bk * 4 + s
                        stat = in_t[:, g, :, :]       # (128, 4, 32) -> M=128
                        mov = in_t[:, g, :, 0:F]      # (128, 4, 27) -> N=108
                        nc.tensor.matmul(ps[:, s, :], stat, mov, start=True, stop=True)
                    for q in range(4):
                        src = ps[32 * q:32 * q + F, :, 27 * q:27 * q + F]
                        dst = out_t[32 * q:32 * q + F, bk * 4:bk * 4 + 4, :]
                        if q < 2:
                            nc.vector.tensor_copy(dst, src)
                        else:
                            nc.scalar.copy(dst, src)
                nc.sync.dma_start(y[:, c, :, :], out_t[:, :, :])

    nc.compile()
    return nc


def _get_compiled():
    global _compiled
    if _compiled is None:
        _compiled = _build()
    return _compiled


def kernel(embeddings: np.ndarray) -> np.ndarray:
    from concourse.bass_utils import run_bass_kernel_spmd

    nc = _get_compiled()

    e = np.asarray(embeddings, dtype=np.float32)
    et = np.ascontiguousarray(e.transpose(2, 0, 1))  # (128, B, 27)
    in_maps = [
        {"et": np.ascontiguousarray(et[:, c * BLOC:(c + 1) * BLOC, :])}
        for c in range(NCORES)
    ]
    res = run_bass_kernel_spmd(nc, in_maps, core_ids=list(range(NCORES)))

    out = np.empty((B, len(_TRIU_I)), dtype=np.float32)
    for c in range(NCORES):
        yv = res.results[c]["y"]  # (128, NCHUNK, NGRP, 27)
        g = yv.reshape(4, 32, NCHUNK, NGRP, F)[:, :F]  # (4, 27, NCHUNK, NGRP, 27)
        g = g.transpose(2, 3, 0, 1, 4).reshape(BLOC, F, F)  # (2048, 27, 27)
        out[c * BLOC:(c + 1) * BLOC] = g[:, _TRIU_I, _TRIU_J]
    return out

